# revision 28
# baseline (speedup 1.0000x reference)
"""LIF spiking-neuron recurrence on Trainium2, 8-core data-parallel SPMD.

Reference recurrence (per neuron, T timesteps):
    h_t = v_{t-1} + (x_t - v_{t-1}) / 2        # TAU = 2.0
    s_t = (h_t >= 1.0)                          # spike
    v_t = (1 - s_t) * h_t                       # hard reset to 0

Kernel computes the algebraically identical scaled form with state
p_t = 2*h_t (bit-exact: *0.5 is exact, the charge add rounds once, the
compare/select are exact):

    p_{t+1} = (p_t < 2) * p_t * 0.5 + x_{t+1}
    s_t     = (p_t >= 2)  as u8

Active design (v14, build_lif_bass_v14): the pipeline is paced by the
x loads (32 MiB/core, ~1456 ns/step of shared DMA), so every engine
must stay under that per-step budget:
  - state: one fused custom DVE op per step (LIF_STATE_ANT:
    reset+decay+charge), run as TWO interleaved half-width chains
    writing slices of one rotation buffer (hides the dependency
    latency between serial steps).
  - fires: most columns fire on the Pool engine as weighted u8 spikes
    (p >= 2)*2^(k//2) written byte-interleaved; a small stripe fires
    on the otherwise-idle ACT engine as sigmoid NOT-spike {0,1}.
  - packing: u16 tensor adds on DVE (2-byte dtype -> DVE 2x mode)
    Horner-fold each 8 steps of weighted lanes into one u16 of two
    spike nibbles, cutting spike stores to ~2.4 MiB/core.
Total DMA ~100.7 us/core runs gapless: head + loads + deferred stores
+ semaphore tail ~= the whole schedule.

Sharding: flatten [B, N] -> 1,048,576 independent neurons, contiguous
1/8 slice per core. Time recurrence stays local per core.
"""

import numpy as np

import concourse.bacc as bacc
import concourse.bass as bass
import concourse.dve_ops as dve_ops
import concourse.mybir as mybir
from concourse.bass_utils import run_bass_kernel_spmd
from concourse.dve_spec import C0, C1, Spec, Src0, Src1, _has_src1
from concourse.dve_spec import lower as dve_lower
from concourse.dve_uop import DveOpSpec
from concourse.tile import TileContext

T = 64
B = 16
N = 65536
P = 128               # SBUF partitions
N_CORES = 8
NEUR = B * N                      # 1048576 neurons
NEUR_PER_CORE = NEUR // N_CORES   # 131072
FD = NEUR_PER_CORE // P           # 1024 fp32 per partition per timestep

# Timesteps batched per DMA transfer (fewer descriptors / less HWDGE+SEQ
# load; transfer bytes unchanged).
NB = 2
X_BUFS = 4   # in-flight input tiles (each NB steps wide)
S_BUFS = 4   # in-flight spike tiles (each NB steps wide)


def _lif_ref(in0, in1, s0, s1, imm2):
    out = (in0 < s0).astype(np.float32) * in0 * np.float32(s1)
    return (out + in1).astype(np.float32)


def register_lif_op():
    """Register the fused LIF state-update op in the concourse custom-DVE
    registry (idempotent): out = (in0 < s0) * in0 * s1 + in1."""
    name = "LIF_STATE_ANT"
    for o in dve_ops.OPS:
        if o.name == name:
            return o
    spec = Spec(
        body=(Src0 < C0) * Src0 * C1 + Src1,
        reference=_lif_ref,
    )
    row = max(dve_ops._SUB_OPCODE_FOR_NAME.values()) + 1
    assert row < 0x20, "custom-DVE opcode rows exhausted"
    dve_ops._SUB_OPCODE_FOR_NAME[name] = row
    shas = {}
    for ver in ("v3", "v4"):
        uops = dve_lower(spec, ver=ver)
        shas[ver] = DveOpSpec(
            name=name, opcode=row, uops=uops, rd1_en=_has_src1(spec)
        ).sha(ver)
    op = dve_ops.DveOp(name, spec, subdim=False, uops_sha=shas)
    dve_ops.OPS.append(op)
    dve_ops.CUSTOM_DVE_SPECS[name] = spec
    return op


def _pack2_ref(in0, in1, s0, s1, imm2):
    return (
        (in0 >= s0).astype(np.float32) + (in1 >= s0).astype(np.float32) * np.float32(s1)
    ).astype(np.float32)


def register_pack_op():
    """Fused 2-step fire+pack: out = (in0 >= s0) + (in1 >= s0)*s1.
    With s1=2 and u8 out this packs two timesteps' spikes into one byte,
    halving spike-store HBM traffic for the packed columns."""
    name = "LIF_PACK2_ANT"
    for o in dve_ops.OPS:
        if o.name == name:
            return o
    spec = Spec(
        body=(Src0 >= C0) + (Src1 >= C0) * C1,
        reference=_pack2_ref,
    )
    row = max(dve_ops._SUB_OPCODE_FOR_NAME.values()) + 1
    assert row < 0x20, "custom-DVE opcode rows exhausted"
    dve_ops._SUB_OPCODE_FOR_NAME[name] = row
    shas = {}
    for ver in ("v3", "v4"):
        uops = dve_lower(spec, ver=ver)
        shas[ver] = DveOpSpec(
            name=name, opcode=row, uops=uops, rd1_en=_has_src1(spec)
        ).sha(ver)
    op = dve_ops.DveOp(name, spec, subdim=False, uops_sha=shas)
    dve_ops.OPS.append(op)
    dve_ops.CUSTOM_DVE_SPECS[name] = spec
    return op


# column split for v11: first PKC columns are bit-packed (2 steps/byte,
# DVE pack op), the remaining PLC fire unpacked on Pool. 492/532 balances
# DVE compute-end (which gates the last stores) against DMA bytes.
PKC = 492
PLC = FD - PKC


def build_lif_bass_v11(
    t_steps: int = T,
    fd: int = FD,
    nb: int = NB,
    x_bufs: int = 8,
    pkc: int = PKC,
) -> bass.Bass:
    """v9 + packed spike output: state update unchanged (fused custom op);
    fire is split into a DVE pack2 op over `pkc` columns (2 steps -> one
    u8, halving those columns' store bytes) and a Pool is_ge over the
    rest. Outputs: s_pk [P, T/2*pkc] u8 (p-major, packed), s_pl
    [P, T*plc] u8 (p-major, plain)."""
    assert t_steps % (2 * nb) == 0
    plc = fd - pkc
    f32 = mybir.dt.float32
    u8 = mybir.dt.uint8
    A = mybir.AluOpType
    lif_op = register_lif_op()
    pack_op = register_pack_op()

    nc = bacc.Bacc(trn_type="TRN2")
    x = nc.dram_tensor("x", [t_steps, P * fd], f32, kind="ExternalInput")
    s_pk = nc.dram_tensor("s_pk", [P, (t_steps // 2) * pkc], u8,
                          kind="ExternalOutput")
    s_pl = nc.dram_tensor("s_pl", [P, t_steps * plc], u8,
                          kind="ExternalOutput")
    xb = x.rearrange("(tb ti) (p f) -> tb p ti f", ti=nb, p=P)
    pkv = s_pk.rearrange("p (tb c) -> p tb c", c=pkc)
    plv = s_pl.rearrange("p (t c) -> p t c", c=plc)

    with TileContext(nc) as tc:
        with (
            tc.tile_pool(name="state", bufs=1) as state,
            tc.tile_pool(name="xin", bufs=x_bufs) as xpool,
        ):
            # 3-deep state rotation: the buffer a state op overwrites was
            # last read two full steps ago, so WAR waits (vs Pool's fire
            # and the pack op) are long satisfied by the time they're checked
            # two independent half-width state chains: the serial custom-DVE
            # ops of chains A and B interleave on the engine, hiding each
            # other's dependency bubbles; rotation depth `rot` lets Pool/ACT
            # fires lag `rot` steps before they stall the chain (WAR)
            hs = half if half else fd
            pA = [state.tile([P, hs], f32, name=f"pa_{i}") for i in range(rot)]
            pB = ([state.tile([P, fd - half], f32, name=f"pb_{i}") for i in range(rot)]
                  if half else None)
            spk_all = state.tile([P, t_steps // 2, pkc], u8, name="spk_all")
            spl_all = state.tile([P, t_steps, plc], u8, name="spl_all")
            dmy = state.tile([P, 1], f32, name="dmy")
            cur = pbufs[0]

            xt_b = None
            x_tiles = []
            for t in range(t_steps):
                tb, ti = divmod(t, nb)
                if ti == 0:
                    xt_b = xpool.tile([P, nb, fd], f32, tag="x", name=f"x_{tb}")
                    if tb == 0:
                        for k in range(nb):
                            nc.sync.dma_start(
                                out=xt_b[:, k : k + 1, :],
                                in_=xb[0, :, k : k + 1, :],
                            )
                    else:
                        nc.sync.dma_start(out=xt_b, in_=xb[tb])
                    x_tiles.append(xt_b)
                if t == 0:
                    # v_{-1} = 0, so p_0 = x_0 exactly: no state op needed —
                    # step 0 reads the x tile directly, shortening the
                    # serial DVE chain (which gates the program end)
                    cur = x_tiles[0][:, 0, :]
                    nc.gpsimd.tensor_scalar(
                        spl_all[:, 0, :], cur[:, pkc:], 2.0, None, A.is_ge
                    )
                    continue
                nxt = pbufs[t % 3]
                # state: nxt = (cur < 2)*cur*0.5 + x_t   (fused reset+charge)
                nc.vector._custom_dve(
                    lif_op, out=nxt, in0=cur, in1=xt_b[:, ti, :], s0=2.0, s1=0.5
                )
                # fire, plain columns on Pool
                nc.gpsimd.tensor_scalar(
                    spl_all[:, t, :], nxt[:, pkc:], 2.0, None, A.is_ge
                )
                if t % 2 == 1:
                    # fire+pack both steps' packed columns: cur still holds
                    # step t-1's state (x_0 itself for the first pack),
                    # nxt holds step t's
                    nc.vector._custom_dve(
                        pack_op, out=spk_all[:, t // 2, :],
                        in0=cur[:, :pkc], in1=nxt[:, :pkc], s0=2.0, s1=2.0,
                    )
                cur = nxt

            # gate the store stream behind the final load (ACT FIFO; SP
            # stores are behind the ins on SP's FIFO already)
            nc.scalar.copy(dmy, x_tiles[-1][:, nb - 1, :1])

            # stores: descending chunk sizes — big chunks while fires are
            # plentiful, tiny chunks only at the fire-gated very end —
            # emitted in gate order, alternating SP/ACT issue queues so
            # per-DMA issue overhead overlaps transfers
            pl_chunks = [(0, 16), (16, 16), (32, 16), (48, 8),
                         (56, 7), (63, 1)]
            pk_chunks = [(0, 8), (8, 8), (16, 8), (24, 4),
                         (28, 3), (31, 1)]
            stores = [("pl", o, w, o + w - 1) for o, w in pl_chunks]
            stores += [("pk", o, w, 2 * (o + w) - 1) for o, w in pk_chunks]
            stores.sort(key=lambda r: r[3])
            q = [nc.scalar, nc.sync]
            for j, (kind, o, w, gate) in enumerate(stores):
                eng = q[j % 2]
                if kind == "pl":
                    eng.dma_start(
                        out=plv[:, o : o + w, :], in_=spl_all[:, o : o + w, :]
                    )
                else:
                    eng.dma_start(
                        out=pkv[:, o : o + w, :], in_=spk_all[:, o : o + w, :]
                    )

    nc.finalize()
    return nc


def build_lif_bass_v9(
    t_steps: int = T,
    fd: int = FD,
    nb: int = NB,
    x_bufs: int = X_BUFS,
    s_bufs: int = S_BUFS,
    fire_dve_cols: int = FD,
) -> bass.Bass:
    """Per-core kernel: x [t_steps, P*fd] f32 -> s [t_steps, P*fd] u8.

    Per step: one fused custom-DVE state op + one 2x-mode tensor_scalar
    fire. State ping-pongs between two SBUF tiles so the fire of step t
    and the state op of step t+1 never alias.
    """
    assert t_steps % nb == 0
    f32 = mybir.dt.float32
    u8 = mybir.dt.uint8
    A = mybir.AluOpType
    lif_op = register_lif_op()

    nc = bacc.Bacc(trn_type="TRN2")
    x = nc.dram_tensor("x", [t_steps, P * fd], f32, kind="ExternalInput")
    s = nc.dram_tensor("s", [t_steps, P * fd], u8, kind="ExternalOutput")
    xb = x.rearrange("(tb ti) (p f) -> tb p ti f", ti=nb, p=P)

    # Spikes accumulate in ONE big SBUF tile (64 KiB/partition) and are
    # stored to HBM only after every x load has issued: total DMA traffic
    # (116.5us) exceeds DVE compute (111us), so the schedule end is
    # DMA-bound, and any store that interleaves with the input stream
    # delays x arrivals and stalls compute. A tiny ACT op that reads the
    # last x tile gates the store stream (ACT's queue is FIFO) behind the
    # final load; the tail is stored per-step so the last, fire-gated
    # store is small.
    nbo = 8
    sb = s.rearrange("(tb ti) (p f) -> tb p ti f", ti=nbo, p=P)

    with TileContext(nc) as tc:
        with (
            tc.tile_pool(name="state", bufs=1) as state,
            tc.tile_pool(name="xin", bufs=x_bufs) as xpool,
        ):
            pa = state.tile([P, fd], f32, name="p_a")
            pb = state.tile([P, fd], f32, name="p_b")
            s_all = state.tile([P, t_steps, fd], u8, name="s_all")
            dmy = state.tile([P, 1], f32, name="dmy")
            nc.vector.memset(pa, 0.0)
            cur = pa

            xt_b = None
            x_tiles = []
            for t in range(t_steps):
                tb, ti = divmod(t, nb)
                if ti == 0:
                    xt_b = xpool.tile([P, nb, fd], f32, tag="x", name=f"x_{tb}")
                    if tb == 0:
                        # split the first load per-step so compute starts
                        # after 1/nb of the transfer
                        for k in range(nb):
                            nc.sync.dma_start(
                                out=xt_b[:, k : k + 1, :],
                                in_=xb[0, :, k : k + 1, :],
                            )
                    else:
                        nc.sync.dma_start(out=xt_b, in_=xb[tb])
                    x_tiles.append(xt_b)
                nxt = pb if cur is pa else pa
                # state: nxt = (cur < 2)*cur*0.5 + x_t   (fused reset+charge)
                nc.vector._custom_dve(
                    lif_op, out=nxt, in0=cur, in1=xt_b[:, ti, :], s0=2.0, s1=0.5
                )
                # fire: s_t = (nxt >= 2) as u8 — column-split between DVE
                # and the otherwise-idle Pool engine so DVE (the pacing
                # engine) finishes before the DMA window closes
                fc = fire_dve_cols
                nc.vector.tensor_scalar(
                    s_all[:, t, :fc], nxt[:, :fc], 2.0, None, A.is_ge
                )
                if fc < fd:
                    nc.gpsimd.tensor_scalar(
                        s_all[:, t, fc:], nxt[:, fc:], 2.0, None, A.is_ge
                    )
                cur = nxt

            # gate: ACT reads the last x tile, so the stores queued behind
            # this on ACT's FIFO cannot start before the final load landed
            # (stores on SP's queue are gated for free: FIFO behind the ins)
            nc.scalar.copy(dmy, x_tiles[-1][:, nb - 1, :1])

            tail = 8
            for j, o in enumerate(range(0, t_steps - tail, nbo)):
                eng = nc.scalar if j % 2 == 0 else nc.sync
                eng.dma_start(out=sb[o // nbo], in_=s_all[:, o : o + nbo, :])
            for j, t in enumerate(range(t_steps - tail, t_steps)):
                tb, ti = divmod(t, nbo)
                # alternate issue queues so the ~720ns per-DMA issue path
                # overlaps across the small tail stores
                eng = nc.scalar if j % 2 == 0 else nc.sync
                eng.dma_start(
                    out=sb[tb, :, ti : ti + 1, :],
                    in_=s_all[:, t : t + 1, :],
                )

    nc.finalize()
    return nc


def build_lif_bass_v14(
    t_steps: int = T,
    fd: int = FD,
    nb: int = NB,
    x_bufs: int = 8,
    cp: int = 920,    # pool-fired columns (weighted u8 spikes, bit-packed)
    half: int = 512,  # state-chain split point (two interleaved DVE chains)
    rot: int = 4,     # state-buffer rotation depth
    batches: tuple = tuple((g, g + 1) for g in range(8)),  # combine batching
    tail_split: bool = False,  # pipeline the last group in column halves
    n_fold: int = 0,  # early K groups folded u16->u8 in DVE's tail window
    _ablate: frozenset = frozenset(),
):
    """v14: load-paced pipeline; fires off the DVE; 8-steps-per-u16 output.

    The pipeline is paced by the x loads (1456 ns/step of DMA); every
    engine must stay under that per-step budget or the whole chain lags:
      - state: TWO interleaved half-width custom-DVE chains writing
        disjoint slices of one rotation buffer. Interleaving hides each
        chain's ~195 ns dependency latency behind the other's execution
        (~1190 ns/step vs 1322 single-chain); one shared tile keeps the
        fires single-op.
      - fires: cols [0, cp) on Pool as weighted u8 (p >= 2)*2^(k//2),
        byte-interleaved pairs (step 2m -> lane0 of pair-tile m, 2m+1 ->
        lane1); cols [cp, fd) on ACT as sigmoid NOT-spike {0,1} u8
        (exact even at p == 2.0), stored unpacked.
      - combines: u16 tree-adds on DVE (2x mode): tA = T0+T1 (ready
        after step 8g+3), tB = T2+T3, K = tA+tB -> one u16 of two spike
        nibbles (lo = even steps, hi = odd steps) per col per 8 steps.
        Early groups batch 4-wide to amortize instruction overhead; the
        last groups run solo so the tail after the final fire is short.
      - stores: ~2.6 MiB/core, gated by tile_wait_until past the load
        stream so the scheduler cannot slot them into the load window.
    """
    assert t_steps % 8 == 0
    g_tot = t_steps // 8
    ca = fd - cp
    f32 = mybir.dt.float32
    u8 = mybir.dt.uint8
    u16 = mybir.dt.uint16
    A = mybir.AluOpType
    SIG = mybir.ActivationFunctionType.Sigmoid
    S = float(2 ** 40)
    lif_op = register_lif_op()
    STEP_MS = 1456e-6   # per-step load pace in tile_wait ms units

    nc = bacc.Bacc(trn_type="TRN2")
    x = nc.dram_tensor("x", [t_steps, P * fd], f32, kind="ExternalInput")
    s16 = nc.dram_tensor("s16", [P, (g_tot - n_fold) * cp], u16,
                         kind="ExternalOutput")
    s8k = (nc.dram_tensor("s8k", [P, n_fold * cp], u8, kind="ExternalOutput")
           if n_fold else None)
    s8a = nc.dram_tensor("s8a", [P, t_steps * ca], u8, kind="ExternalOutput")
    xb = x.rearrange("(tb ti) (p f) -> tb p ti f", ti=nb, p=P)
    sv = s16.rearrange("p (g c) -> p g c", c=cp)
    fv = s8k.rearrange("p (g c) -> p g c", c=cp) if n_fold else None
    av = s8a.rearrange("p (t c) -> p t c", c=ca)

    with TileContext(nc) as tc:
        with (
            tc.tile_pool(name="state", bufs=1) as state,
            tc.tile_pool(name="xin", bufs=x_bufs) as xpool,
        ):
            pbufs = [state.tile([P, fd], f32, name=f"p_{i}") for i in range(rot)]
            W = state.tile([P, g_tot, 4, cp, 2], u8, name="W")
            K = state.tile([P, g_tot, cp], u16, name="K")
            Au = state.tile([P, t_steps, ca], u8, name="Au")
            tA = state.tile([P, 4, cp], u16, name="tA")
            tB = state.tile([P, 4, cp], u16, name="tB")
            Kb = state.tile([P, max(n_fold, 1), cp], u8, name="Kb")
            K8 = K.bitcast(u8).rearrange(
                "p g (c two) -> p g c two", two=2)   # [P, g_tot, cp, 2]
            bsc = state.tile([P, 1], f32, name="bsc")
            dmy = state.tile([P, 1], f32, name="dmy")
            nc.vector.memset(bsc, 2.0 * S)
            W16 = W.bitcast(u16)   # [P, g_tot, 4, cp]

            pending = []

            def combine_ops_for(lo, hi, c0=0, c1=None):
                # pair-tile m of group g holds steps (8g+2m, 8g+2m+1) as
                # u16 byte lanes; tA/tB/K tree-add whole batches [lo, hi)
                n = hi - lo
                gs = slice(lo, hi)
                c1 = cp if c1 is None else c1
                cs = slice(c0, c1)
                # Horner chain: each add needs only 2 more steps' fires,
                # so just ONE add trails the group's final fire
                yield 8 * (hi - 1) + 6, lambda: nc.vector.tensor_tensor(
                    tA[:, :n, cs], W16[:, gs, 0, cs], W16[:, gs, 1, cs], A.add)
                yield 8 * (hi - 1) + 8, lambda: nc.vector.tensor_tensor(
                    tB[:, :n, cs], tA[:, :n, cs], W16[:, gs, 2, cs], A.add)
                yield 8 * (hi - 1) + 10, lambda: nc.vector.tensor_tensor(
                    K[:, gs, cs], tB[:, :n, cs], W16[:, gs, 3, cs], A.add)

            batch_of = {hi - 1: (lo, hi) for lo, hi in batches}

            curA, curB = None, None
            xt_b = None
            x_tiles = []
            for t in range(t_steps):
                tb, ti = divmod(t, nb)
                g, k = divmod(t, 8)
                kk, lane = k // 2, k % 2
                if ti == 0:
                    xt_b = xpool.tile([P, nb, fd], f32, tag="x", name=f"x_{tb}")
                    if tb == 0 or tb == t_steps // nb - 1:
                        # first tile: per-step loads on BOTH issue queues
                        # (parallel issue paths shorten the head); last
                        # tile: step T-2's state needn't wait for step
                        # T-1's bytes (shorter tail chain)
                        qs = [nc.sync, nc.scalar] if tb == 0 else [nc.sync]
                        for j in range(nb):
                            qs[j % len(qs)].dma_start(
                                out=xt_b[:, j : j + 1, :],
                                in_=xb[tb, :, j : j + 1, :],
                            )
                    else:
                        nc.sync.dma_start(out=xt_b, in_=xb[tb])
                    x_tiles.append(xt_b)
                if t == 0:
                    # p_0 = x_0 exactly (v_{-1} = 0): read the x tile directly
                    curA = x_tiles[0][:, 0, :half]
                    curB = x_tiles[0][:, 0, half:]
                    full = x_tiles[0][:, 0, :]
                elif "state" not in _ablate:
                    buf = pbufs[t % rot]
                    nc.vector._custom_dve(
                        lif_op, out=buf[:, :half], in0=curA,
                        in1=xt_b[:, ti, :half], s0=2.0, s1=0.5,
                    )
                    nc.vector._custom_dve(
                        lif_op, out=buf[:, half:], in0=curB,
                        in1=xt_b[:, ti, half:], s0=2.0, s1=0.5,
                    )
                    curA, curB = buf[:, :half], buf[:, half:]
                    full = buf
                if pending and "combines" not in _ablate and t >= pending[0][0]:
                    tgt, op = pending.pop(0)
                    with tc.tile_wait_until(tgt * STEP_MS):
                        op()
                if "fires" not in _ablate and "pfires" not in _ablate:
                    wgt = float(2 ** (kk + 4 * lane))
                    if tail_split and g == g_tot - 1:
                        # last group: fire in halves so the left half's
                        # combines+store can start while the right fires
                        ch = (cp // 2) & ~15
                        nc.gpsimd.tensor_scalar(
                            W[:, g, kk, :ch, lane], full[:, :ch],
                            2.0, wgt, A.is_ge, A.mult,
                        )
                        nc.gpsimd.tensor_scalar(
                            W[:, g, kk, ch:cp, lane], full[:, ch:cp],
                            2.0, wgt, A.is_ge, A.mult,
                        )
                    else:
                        nc.gpsimd.tensor_scalar(
                            W[:, g, kk, :, lane], full[:, :cp],
                            2.0, wgt, A.is_ge, A.mult,
                        )
                if "fires" not in _ablate and "afires" not in _ablate:
                    nc.scalar.activation(
                        Au[:, t, :], full[:, cp:], SIG, bias=bsc[:, :],
                        scale=-S,
                    )
                if k == 7 and g in ():
                    pass
                if (t % 8 == 7) and ((t // 8) in [hi - 1 for _, hi in batches]):
                    pass
                if k == 7:
                    for lo_hi in [b for b in batches if b[1] - 1 == g]:
                        if tail_split and lo_hi == (g_tot - 1, g_tot):
                            ch = (cp // 2) & ~15
                            pending.extend(combine_ops_for(*lo_hi, 0, ch))
                            pending.extend(combine_ops_for(*lo_hi, ch, cp))
                        else:
                            pending.extend(combine_ops_for(*lo_hi))
            if "combines" not in _ablate:
                for tgt, op in pending:
                    op()
                # true-pack8 folds for early groups: byte = lo + hi (the
                # two lanes carry disjoint nibbles); runs in DVE's idle
                # window after the state chain ends
                for g in range(n_fold):
                    with tc.tile_wait_until((t_steps + 1 + g) * STEP_MS):
                        nc.vector.tensor_tensor(
                            Kb[:, g, :], K8[:, g, :, 0], K8[:, g, :, 1],
                            A.add,
                        )

            # gate: stores must not enter the load window; wait_until puts
            # them past the final load regardless of scheduler choices
            nc.scalar.copy(dmy, x_tiles[-1][:, nb - 1, :1])
            if "stores" not in _ablate:
                # readiness-ordered: everything data-ready early goes first
                # on each queue; the two latest producers (ACT fire 63 ->
                # av tail, Pool fire 63 -> g7 combine -> K tail) sit last
                # on separate queues so neither waits behind the other
                nf = n_fold
                with tc.tile_wait_until((t_steps + 2) * STEP_MS):
                    nc.scalar.dma_start(out=sv[:, 0 : 4 - nf, :],
                                        in_=K[:, nf:4, :])
                    nc.sync.dma_start(out=av[:, 0:16, :], in_=Au[:, 0:16, :])
                    nc.sync.dma_start(out=av[:, 16:32, :], in_=Au[:, 16:32, :])
                    nc.scalar.dma_start(out=sv[:, 4 - nf : 6 - nf, :],
                                        in_=K[:, 4:6, :])
                    nc.sync.dma_start(out=av[:, 32:48, :], in_=Au[:, 32:48, :])
                    nc.scalar.dma_start(out=sv[:, 6 - nf : 7 - nf, :],
                                        in_=K[:, 6:7, :])
                    nc.sync.dma_start(out=av[:, 48:64, :], in_=Au[:, 48:64, :])
                    if nf:
                        nc.sync.dma_start(out=fv[:, :, :], in_=Kb[:, :nf, :])
                    if tail_split:
                        ch = (cp // 2) & ~15
                        nc.scalar.dma_start(out=sv[:, 7 - nf : 8 - nf, :ch],
                                            in_=K[:, 7:8, :ch])
                        nc.sync.dma_start(out=sv[:, 7 - nf : 8 - nf, ch:],
                                          in_=K[:, 7:8, ch:])
                    else:
                        nc.scalar.dma_start(out=sv[:, 7 - nf : 8 - nf, :],
                                            in_=K[:, 7:8, :])

    nc.finalize()
    return nc


_NC_CACHE: dict = {}

DESIGN = "v14"   # "v9" | "v11" (2-step packed) | "v14" (8-step packed)


V14_CP = 920


def _get_nc():
    if DESIGN not in _NC_CACHE:
        if DESIGN == "v14":
            _NC_CACHE[DESIGN] = build_lif_bass_v14(cp=V14_CP)
        elif DESIGN == "v11":
            _NC_CACHE[DESIGN] = build_lif_bass_v11()
        else:
            _NC_CACHE[DESIGN] = build_lif_bass_v9(x_bufs=8, fire_dve_cols=512)
    return _NC_CACHE[DESIGN]


def kernel(x: np.ndarray) -> np.ndarray:
    assert x.shape == (T, B, N), x.shape
    x = np.ascontiguousarray(x, dtype=np.float32)
    xf = x.reshape(T, NEUR)

    in_maps = []
    for c in range(N_CORES):
        lo = c * NEUR_PER_CORE
        shard = np.ascontiguousarray(xf[:, lo : lo + NEUR_PER_CORE])
        in_maps.append({"x": shard})

    nc = _get_nc()
    res = run_bass_kernel_spmd(nc, in_maps, core_ids=list(range(N_CORES)))

    out = np.empty((T, NEUR), dtype=np.float32)
    for c in range(N_CORES):
        lo = c * NEUR_PER_CORE
        r = res.results[c]
        if DESIGN == "v14":
            # bulk: u16 = (lo byte: even steps' nibble, hi byte: odd):
            # bit j of lo = spike at 8g+2j, of hi = 8g+2j+1
            kv = r["s16"].reshape(P, T // 8, V14_CP)
            # lane0 weights 1..8 (bits 0-3), lane1 weights 16..128 (bits
            # 4-7, pre-shifted): the two bytes hold disjoint bit sets
            byte = ((kv | (kv >> 8)) & 0xFF).astype(np.uint8)
            bits = (byte[:, :, None, :] >> np.arange(8, dtype=np.uint8)[None, None, :, None]) & 1
            # bit index b -> step-in-group perm[b]
            perm = np.array([0, 2, 4, 6, 1, 3, 5, 7])
            steps = np.empty((P, T // 8, 8, V14_CP), dtype=np.uint8)
            steps[:, :, perm, :] = bits
            sc = np.empty((T, P, FD), dtype=np.float32)
            sc[:, :, :V14_CP] = steps.transpose(1, 2, 0, 3).reshape(T, P, V14_CP)
            sc[:, :, V14_CP:] = 1.0 - r["s8a"].reshape(P, T, FD - V14_CP).transpose(1, 0, 2)
            out[:, lo : lo + NEUR_PER_CORE] = sc.reshape(T, NEUR_PER_CORE)
        elif DESIGN == "v11":
            # reassemble: packed columns carry 2 steps/byte (bit0 = even
            # step, bit1 = odd step), plain columns are u8 {0,1}
            sc = np.empty((T, P, FD), dtype=np.float32)
            pk = r["s_pk"].reshape(P, T // 2, PKC)
            sc[0::2, :, :PKC] = (pk & 1).transpose(1, 0, 2)
            sc[1::2, :, :PKC] = (pk >> 1).transpose(1, 0, 2)
            pl = r["s_pl"].reshape(P, T, PLC)
            sc[:, :, PKC:] = pl.transpose(1, 0, 2)
            out[:, lo : lo + NEUR_PER_CORE] = sc.reshape(T, NEUR_PER_CORE)
        else:
            out[:, lo : lo + NEUR_PER_CORE] = r["s"].astype(np.float32)
    return out.reshape(T, B, N)

